# revision 1
# baseline (speedup 1.0000x reference)
"""MoE (top-1 routing, E=8 experts) Trainium2 kernel.

Strategy (expert-parallel across 8 NeuronCores):
  - Routing (softmax/argmax/capacity) is computed on host with jax-on-CPU,
    replicating the reference computation op-for-op so expert assignment
    matches bit-exactly.
  - Dispatch (the "all-to-all") happens host-side while building per-core
    inputs: core e receives the (<=2048) tokens routed to expert e, already
    gathered, scaled by gate probability, and transposed to [D, cap].
  - Each core runs Y_e = relu(Xe @ W1_e) @ W2_e as a dense FFN in fp32r
    (full-rate PE matmul, ~1e-4 relative error).
  - Combine: host scatters each core's [cap, D] output back to token order.
"""

import os
import sys

for _p in ("/opt/trn_rl_repo",):
    if os.path.isdir(_p) and _p not in sys.path:
        sys.path.insert(0, _p)

import numpy as np

B, S, D, F, E = 8, 2048, 1024, 4096, 8
T = B * S
CAP = T // E  # 2048, capacity_factor 1.0

F_BLK = 512          # F columns per outer block
N_FBLK = F // F_BLK  # 8
N_DC = D // 128      # 8 contraction chunks for GEMM1
N_FC = F_BLK // 128  # 4 contraction chunks for GEMM2 per block
N_TG = CAP // 128    # 16 token groups
N_TC = CAP // 512    # 4 token columns


def _build_nc():
    import concourse.bacc as bacc
    import concourse.mybir as mybir
    from concourse.bass import ds
    from concourse.tile import TileContext

    f32 = mybir.dt.float32
    f32r = mybir.dt.float32r

    nc = bacc.Bacc("TRN2", target_bir_lowering=False, debug=False, num_devices=E)

    xeT = nc.dram_tensor("xeT", [D, CAP], f32r, kind="ExternalInput")
    w1 = nc.dram_tensor("w1", [D, F], f32r, kind="ExternalInput")
    w2 = nc.dram_tensor("w2", [F, D], f32r, kind="ExternalInput")
    y = nc.dram_tensor("y", [CAP, D], f32, kind="ExternalOutput")

    x_r = xeT.ap().rearrange("(dc p) t -> p dc t", p=128)
    w1_r = w1.ap().rearrange("(dc p) f -> p dc f", p=128)
    w2_r = w2.ap().rearrange("(r p) d -> p r d", p=128)
    y_r = y.ap().rearrange("(tg p) d -> p tg d", p=128)

    with TileContext(nc) as tc:
        with (
            tc.tile_pool(name="xpool", bufs=1) as xpool,
            tc.tile_pool(name="ypool", bufs=1) as ypool,
            tc.tile_pool(name="wpool", bufs=1) as wpool,
            tc.tile_pool(name="hpool", bufs=1) as hpool,
            tc.tile_pool(name="psh", bufs=3, space="PSUM") as psh,
            tc.tile_pool(name="psy", bufs=5, space="PSUM") as psy,
        ):
            # PE warmup: dependency-free fp32 matmuls keep the PE busy while
            # the first DMAs land, so HAM un-throttles to 2.4 GHz before the
            # real matmul stream starts. (memset supports fp32, and warmup
            # only needs PE activity — precision is irrelevant.)
            warm_sb = xpool.tile([128, 384], f32)
            nc.gpsimd.memset(warm_sb, 0)
            # Warmup psum shares the GEMM2 pool's slots (no dedicated bank).
            for _ in range(19):
                pwarm = psy.tile([128, 512], f32, tag="py")
                nc.tensor.matmul(
                    pwarm[:, :256], warm_sb[:, :128], warm_sb[:, ds(128, 256)],
                    start=True, stop=True,
                )

            # Weights stream on the sync (HWDGE) queue; xeT streams on the
            # gpsimd (SWDGE) queue in parallel, in 8 chunks so the first
            # GEMM1 groups can start after ~1 MB has landed.
            xeT_sb = xpool.tile([128, N_DC, CAP], f32r)
            for tcix in range(2 * N_TC):
                nc.gpsimd.dma_start(
                    out=xeT_sb[:, :, ds(tcix * 256, 256)],
                    in_=x_r[:, :, ds(tcix * 256, 256)],
                )

            y_sb = ypool.tile([128, N_TG, D], f32)

            for fo in range(N_FBLK):
                w1t = wpool.tile([128, N_DC, F_BLK], f32r, tag="w1t")
                if fo == 0:
                    # dc-split so the first matmuls start on partial data
                    # while the rest of w1/xeT is still in flight.
                    for dc in range(N_DC):
                        nc.sync.dma_start(
                            out=w1t[:, dc, :],
                            in_=w1_r[:, dc, ds(fo * F_BLK, F_BLK)],
                        )
                else:
                    nc.sync.dma_start(out=w1t, in_=w1_r[:, :, ds(fo * F_BLK, F_BLK)])
                w2t = wpool.tile([128, N_FC, D], f32r, tag="w2t")
                nc.sync.dma_start(out=w2t, in_=w2_r[:, ds(fo * N_FC, N_FC), :])

                hT = hpool.tile([128, N_FC, CAP], f32r)
                # GEMM1: hT[f, t] = relu(sum_d W1[d, f] * XeT[d, t])
                # tc-outer so groups only need the xeT chunks that have
                # arrived; fo==0 walks 256-token columns to match the
                # streaming xeT arrival.
                tok_cols = 2 * N_TC if fo == 0 else N_TC
                tok_w = CAP // tok_cols
                for tcix in range(tok_cols):
                    for fc in range(N_FC):
                        ph = psh.tile([128, 512], f32, tag="ph")
                        for dc in range(N_DC):
                            nc.tensor.matmul(
                                ph[:, :tok_w],
                                w1t[:, dc, ds(fc * 128, 128)],
                                xeT_sb[:, dc, ds(tcix * tok_w, tok_w)],
                                start=(dc == 0),
                                stop=(dc == N_DC - 1),
                            )
                        nc.scalar.activation(
                            hT[:, fc, ds(tcix * tok_w, tok_w)],
                            ph[:, :tok_w],
                            mybir.ActivationFunctionType.Relu,
                        )

                # GEMM2: y[t, d] += sum_f hT[f, t] * W2[f, d]
                for tg in range(N_TG):
                    for dh in range(2):
                        py = psy.tile([128, 512], f32, tag="py")
                        for fc in range(N_FC):
                            nc.tensor.matmul(
                                py,
                                hT[:, fc, ds(tg * 128, 128)],
                                w2t[:, fc, ds(dh * 512, 512)],
                                start=(fc == 0),
                                stop=(fc == N_FC - 1),
                            )
                        dst = y_sb[:, tg, ds(dh * 512, 512)]
                        if fo == 0:
                            nc.vector.tensor_copy(dst, py)
                        else:
                            nc.vector.tensor_add(dst, dst, py)

            for tg in range(N_TG):
                nc.sync.dma_start(out=y_r[:, tg, :], in_=y_sb[:, tg, :])

    nc.compile()
    return nc


_NC = None


def _get_nc():
    global _NC
    if _NC is None:
        _NC = _build_nc()
    return _NC


def _route(xf, Wr):
    """Replicates the reference routing (jax-on-CPU, op-for-op) so that
    expert assignment matches the fp32 reference bit-exactly."""
    try:
        import jax
        import jax.numpy as jnp

        cpu = jax.local_devices(backend="cpu")[0]
        with jax.default_device(cpu):
            xj = jnp.asarray(xf, dtype=jnp.float32)
            wj = jnp.asarray(Wr, dtype=jnp.float32)
            probs = jax.nn.softmax(xj @ wj, axis=-1)
            eidx_j = jnp.argmax(probs, axis=-1)
            p_tok_j = jnp.take_along_axis(probs, eidx_j[:, None], axis=1)[:, 0]
            eidx = np.asarray(eidx_j)
            p_tok = np.asarray(p_tok_j)
    except Exception:
        # numpy fallback (fp32, same math; argmax ties broken identically
        # by first-max)
        logits = xf.astype(np.float32) @ Wr.astype(np.float32)
        lmax = logits.max(axis=-1, keepdims=True)
        ex = np.exp(logits - lmax)
        probs = ex / ex.sum(axis=-1, keepdims=True)
        eidx = np.argmax(probs, axis=-1)
        p_tok = probs[np.arange(T), eidx]

    # Integer capacity logic (exact) in numpy.
    onehot = np.zeros((T, E), dtype=np.int64)
    onehot[np.arange(T), eidx] = 1
    rank = np.cumsum(onehot, axis=0) - onehot
    rank = rank[np.arange(T), eidx]  # earlier same-expert tokens
    keep = rank < CAP

    dispatch = np.zeros((E, CAP), dtype=np.int64)
    valid = np.zeros((E, CAP), dtype=bool)
    kept = np.nonzero(keep)[0]
    dispatch[eidx[kept], rank[kept]] = kept
    valid[eidx[kept], rank[kept]] = True
    return dispatch, valid, p_tok


def kernel(x, Wr, W1, W2):
    from concourse.bass_utils import run_bass_kernel_spmd

    x = np.asarray(x, dtype=np.float32)
    Wr = np.asarray(Wr, dtype=np.float32)
    W1 = np.asarray(W1, dtype=np.float32)
    W2 = np.asarray(W2, dtype=np.float32)

    xf = x.reshape(T, D)
    dispatch, valid, p_tok = _route(xf, Wr)

    in_maps = []
    for e in range(E):
        scale = np.where(valid[e], p_tok[dispatch[e]], 0.0).astype(np.float32)
        xe = xf[dispatch[e]] * scale[:, None]  # [CAP, D]; relu(s*x@W1)@W2 = s*y
        in_maps.append({
            "xeT": np.ascontiguousarray(xe.T),
            "w1": np.ascontiguousarray(W1[e]),
            "w2": np.ascontiguousarray(W2[e]),
        })

    nc = _get_nc()
    res = run_bass_kernel_spmd(nc, in_maps, core_ids=list(range(E)))

    yf = np.zeros((T, D), dtype=np.float32)
    for e in range(E):
        ye = res.results[e]["y"]
        m = valid[e]
        yf[dispatch[e][m]] = ye[m]
    return yf.reshape(B, S, D)



# revision 2
# speedup vs baseline: 1.0490x; 1.0490x over previous
"""MoE (top-1 routing, E=8 experts) Trainium2 kernel.

Strategy (expert-parallel across 8 NeuronCores):
  - Routing (softmax/argmax/capacity) is computed on host with jax-on-CPU,
    replicating the reference computation op-for-op so expert assignment
    matches bit-exactly.
  - Dispatch (the "all-to-all") happens host-side while building per-core
    inputs: core e receives the (<=2048) tokens routed to expert e, already
    gathered, scaled by gate probability, transposed to [D, cap], and cast
    to fp16 (same PE rate as fp32r, half the DMA bytes, FWL weight loads).
  - Each core runs Y_e = relu(Xe @ W1_e) @ W2_e as a dense FFN with all
    weights + tokens resident in SBUF. GEMM2 partials per F-block are
    evacuated as fp16 and summed on host (frees SBUF + vector engine,
    streams the output DMA throughout the kernel instead of a tail burst).
  - Combine: host sums the 8 F-block partials, scatters back to token order.
"""

import os
import sys

for _p in ("/opt/trn_rl_repo",):
    if os.path.isdir(_p) and _p not in sys.path:
        sys.path.insert(0, _p)

import numpy as np

B, S, D, F, E = 8, 2048, 1024, 4096, 8
T = B * S
CAP = T // E  # 2048, capacity_factor 1.0

F_BLK = 512          # F columns per outer block
N_FBLK = F // F_BLK  # 8
N_DC = D // 128      # 8 contraction chunks for GEMM1
N_FC = F_BLK // 128  # 4 contraction chunks for GEMM2 per block
N_TG = CAP // 128    # 16 token groups
N_TC = CAP // 512    # 4 token columns


def _build_nc():
    import concourse.bacc as bacc
    import concourse.mybir as mybir
    from concourse.bass import ds
    from concourse.tile import TileContext

    f32 = mybir.dt.float32
    f16 = mybir.dt.float16

    nc = bacc.Bacc("TRN2", target_bir_lowering=False, debug=False, num_devices=E)

    xeT = nc.dram_tensor("xeT", [D, CAP], f16, kind="ExternalInput")
    w1 = nc.dram_tensor("w1", [D, F], f16, kind="ExternalInput")
    w2 = nc.dram_tensor("w2", [F, D], f16, kind="ExternalInput")
    # Per-F-block GEMM2 partials; host sums over axis 0.
    y = nc.dram_tensor("y", [N_FBLK, CAP, D], f16, kind="ExternalOutput")

    x_r = xeT.ap().rearrange("(dc p) t -> p dc t", p=128)
    w1_r = w1.ap().rearrange("(dc p) f -> p dc f", p=128)
    w2_r = w2.ap().rearrange("(fc p) d -> p fc d", p=128)
    y_r = y.ap().rearrange("fo (tg p) d -> p fo tg d", p=128)

    with TileContext(nc) as tc:
        with (
            tc.tile_pool(name="xpool", bufs=1) as xpool,
            tc.tile_pool(name="wpool", bufs=1) as wpool,
            tc.tile_pool(name="hpool", bufs=1) as hpool,
            tc.tile_pool(name="spool", bufs=6) as spool,
            tc.tile_pool(name="psh", bufs=3, space="PSUM") as psh,
            tc.tile_pool(name="psy", bufs=5, space="PSUM") as psy,
        ):
            # PE warmup: dependency-light fp32 matmuls ramp the HAM clock to
            # 2.4 GHz while the first DMAs land. memset on the vector queue
            # starts earlier than gpsimd's.
            warm_sb = xpool.tile([128, 384], f32)
            nc.vector.memset(warm_sb, 0)
            for _ in range(16):
                pwarm = psy.tile([128, 512], f32, tag="py")
                nc.tensor.matmul(
                    pwarm[:, :256], warm_sb[:, :128], warm_sb[:, ds(128, 256)],
                    start=True, stop=True,
                )

            # xeT streams on the gpsimd (SWDGE) queue in 8 chunks so the
            # first GEMM1 groups can start after ~1 MB has landed; weights
            # stream on the sync (HWDGE) queue in parallel.
            xeT_sb = xpool.tile([128, N_DC, CAP], f16)
            for tcix in range(8):
                nc.gpsimd.dma_start(
                    out=xeT_sb[:, :, ds(tcix * 256, 256)],
                    in_=x_r[:, :, ds(tcix * 256, 256)],
                )

            w1_sb = wpool.tile([128, N_DC, F], f16, tag="w1")
            w2_sb = wpool.tile([128, F // 128, D], f16, tag="w2")
            # fo=0 weights first (dc-split so the first GEMM1 chain starts on
            # partial data), then interleave the rest by fo so each block's
            # weights land well before its compute.
            for dc in range(N_DC):
                nc.sync.dma_start(
                    out=w1_sb[:, dc, ds(0, F_BLK)],
                    in_=w1_r[:, dc, ds(0, F_BLK)],
                )
            nc.sync.dma_start(out=w2_sb[:, ds(0, N_FC), :], in_=w2_r[:, ds(0, N_FC), :])
            for fo in range(1, N_FBLK):
                nc.sync.dma_start(
                    out=w1_sb[:, :, ds(fo * F_BLK, F_BLK)],
                    in_=w1_r[:, :, ds(fo * F_BLK, F_BLK)],
                )
                nc.sync.dma_start(
                    out=w2_sb[:, ds(fo * N_FC, N_FC), :],
                    in_=w2_r[:, ds(fo * N_FC, N_FC), :],
                )

            hT = hpool.tile([128, N_FC, CAP], f16)

            for fo in range(N_FBLK):
                # GEMM1: hT[f, t] = relu(sum_d W1[d, f] * XeT[d, t])
                # fo==0 walks 256-token columns to match streaming xeT
                # arrival; later blocks use full 512-wide moving operands.
                tok_cols = 2 * N_TC if fo == 0 else N_TC
                tok_w = CAP // tok_cols
                for tcix in range(tok_cols):
                    for fc in range(N_FC):
                        ph = psh.tile([128, 512], f32, tag="ph")
                        for dc in range(N_DC):
                            nc.tensor.matmul(
                                ph[:, :tok_w],
                                w1_sb[:, dc, ds(fo * F_BLK + fc * 128, 128)],
                                xeT_sb[:, dc, ds(tcix * tok_w, tok_w)],
                                start=(dc == 0),
                                stop=(dc == N_DC - 1),
                            )
                        nc.scalar.activation(
                            hT[:, fc, ds(tcix * tok_w, tok_w)],
                            ph[:, :tok_w],
                            mybir.ActivationFunctionType.Relu,
                        )

                # GEMM2: y_fo[t, d] = sum_f hT[f, t] * W2[f, d]; evacuate each
                # [128, 1024] token-group row as fp16 and stream it out on the
                # gpsimd queue (idle after xeT), summed across fo on host.
                for tg in range(N_TG):
                    stage = spool.tile([128, D], f16, tag="st")
                    for dh in range(2):
                        py = psy.tile([128, 512], f32, tag="py")
                        for fc in range(N_FC):
                            nc.tensor.matmul(
                                py,
                                hT[:, fc, ds(tg * 128, 128)],
                                w2_sb[:, fo * N_FC + fc, ds(dh * 512, 512)],
                                start=(fc == 0),
                                stop=(fc == N_FC - 1),
                            )
                        nc.vector.tensor_copy(stage[:, ds(dh * 512, 512)], py)
                    nc.gpsimd.dma_start(out=y_r[:, fo, tg, :], in_=stage)

    nc.compile()
    return nc


_NC = None


def _get_nc():
    global _NC
    if _NC is None:
        _NC = _build_nc()
    return _NC


def _route(xf, Wr):
    """Replicates the reference routing (jax-on-CPU, op-for-op) so that
    expert assignment matches the fp32 reference bit-exactly."""
    try:
        import jax
        import jax.numpy as jnp

        cpu = jax.local_devices(backend="cpu")[0]
        with jax.default_device(cpu):
            xj = jnp.asarray(xf, dtype=jnp.float32)
            wj = jnp.asarray(Wr, dtype=jnp.float32)
            probs = jax.nn.softmax(xj @ wj, axis=-1)
            eidx_j = jnp.argmax(probs, axis=-1)
            p_tok_j = jnp.take_along_axis(probs, eidx_j[:, None], axis=1)[:, 0]
            eidx = np.asarray(eidx_j)
            p_tok = np.asarray(p_tok_j)
    except Exception:
        # numpy fallback (fp32, same math; argmax ties broken identically
        # by first-max)
        logits = xf.astype(np.float32) @ Wr.astype(np.float32)
        lmax = logits.max(axis=-1, keepdims=True)
        ex = np.exp(logits - lmax)
        probs = ex / ex.sum(axis=-1, keepdims=True)
        eidx = np.argmax(probs, axis=-1)
        p_tok = probs[np.arange(T), eidx]

    # Integer capacity logic (exact) in numpy.
    onehot = np.zeros((T, E), dtype=np.int64)
    onehot[np.arange(T), eidx] = 1
    rank = np.cumsum(onehot, axis=0) - onehot
    rank = rank[np.arange(T), eidx]  # earlier same-expert tokens
    keep = rank < CAP

    dispatch = np.zeros((E, CAP), dtype=np.int64)
    valid = np.zeros((E, CAP), dtype=bool)
    kept = np.nonzero(keep)[0]
    dispatch[eidx[kept], rank[kept]] = kept
    valid[eidx[kept], rank[kept]] = True
    return dispatch, valid, p_tok


def kernel(x, Wr, W1, W2):
    from concourse.bass_utils import run_bass_kernel_spmd

    x = np.asarray(x, dtype=np.float32)
    Wr = np.asarray(Wr, dtype=np.float32)
    W1 = np.asarray(W1, dtype=np.float32)
    W2 = np.asarray(W2, dtype=np.float32)

    xf = x.reshape(T, D)
    dispatch, valid, p_tok = _route(xf, Wr)

    in_maps = []
    for e in range(E):
        scale = np.where(valid[e], p_tok[dispatch[e]], 0.0).astype(np.float32)
        xe = xf[dispatch[e]] * scale[:, None]  # [CAP, D]; relu(s*x@W1)@W2 = s*y
        in_maps.append({
            "xeT": np.ascontiguousarray(xe.T.astype(np.float16)),
            "w1": np.ascontiguousarray(W1[e].astype(np.float16)),
            "w2": np.ascontiguousarray(W2[e].astype(np.float16)),
        })

    nc = _get_nc()
    res = run_bass_kernel_spmd(nc, in_maps, core_ids=list(range(E)))

    yf = np.zeros((T, D), dtype=np.float32)
    for e in range(E):
        ye = res.results[e]["y"].astype(np.float32).sum(axis=0)  # [CAP, D]
        m = valid[e]
        yf[dispatch[e][m]] = ye[m]
    return yf.reshape(B, S, D)


# revision 4
# speedup vs baseline: 1.0619x; 1.0122x over previous
"""MoE (top-1 routing, E=8 experts) Trainium2 kernel.

Strategy (expert-parallel across 8 NeuronCores):
  - Routing (softmax/argmax/capacity) is computed on host with jax-on-CPU,
    replicating the reference computation op-for-op so expert assignment
    matches bit-exactly.
  - Dispatch (the "all-to-all") happens host-side while building per-core
    inputs: core e receives the (<=2048) tokens routed to expert e, already
    gathered, scaled by gate probability, transposed to [D, cap], and cast
    to fp16 (same PE rate as fp32r, half the DMA bytes, FWL weight loads).
  - Each core runs Y_e = relu(Xe @ W1_e) @ W2_e as a dense FFN with all
    weights + tokens resident in SBUF. GEMM2 partials per F-block are
    evacuated as fp16 and summed on host (frees SBUF + vector engine,
    streams the output DMA throughout the kernel instead of a tail burst).
  - Combine: host sums the 8 F-block partials, scatters back to token order.
"""

import os
import sys

for _p in ("/opt/trn_rl_repo",):
    if os.path.isdir(_p) and _p not in sys.path:
        sys.path.insert(0, _p)

import numpy as np

B, S, D, F, E = 8, 2048, 1024, 4096, 8
T = B * S
CAP = T // E  # 2048, capacity_factor 1.0

F_BLK = 512          # F columns per outer block
N_FBLK = F // F_BLK  # 8
N_DC = D // 128      # 8 contraction chunks for GEMM1
N_FC = F_BLK // 128  # 4 contraction chunks for GEMM2 per block
N_TG = CAP // 128    # 16 token groups
N_TC = CAP // 512    # 4 token columns


def _build_nc():
    import concourse.bacc as bacc
    import concourse.mybir as mybir
    from concourse.bass import ds
    from concourse.tile import TileContext

    f32 = mybir.dt.float32
    f16 = mybir.dt.float16

    nc = bacc.Bacc("TRN2", target_bir_lowering=False, debug=False, num_devices=E)

    # Host pre-tiles the inputs so every DMA line is >=4 KB contiguous per
    # partition: w1 [fo, p, dc, 512], w2 [fo, p, fc, 1024], xeT [c, p, dc, 256].
    xeT = nc.dram_tensor("xeT", [8, 128, N_DC, 256], f16, kind="ExternalInput")
    w1 = nc.dram_tensor("w1", [N_FBLK, 128, N_DC, F_BLK], f16, kind="ExternalInput")
    w2 = nc.dram_tensor("w2", [N_FBLK, 128, N_FC, D], f16, kind="ExternalInput")
    # Per-F-block GEMM2 partials; host sums over axis 0.
    y = nc.dram_tensor("y", [N_FBLK, CAP, D], f16, kind="ExternalOutput")

    x_r = xeT.ap().rearrange("c p dc j -> p c dc j")
    w1_r = w1.ap().rearrange("fo p dc j -> p fo dc j")
    w2_r = w2.ap().rearrange("fo p fc d -> p fo fc d")
    y_r = y.ap().rearrange("fo (tg p) d -> p fo tg d", p=128)

    with TileContext(nc) as tc:
        with (
            tc.tile_pool(name="xpool", bufs=1) as xpool,
            tc.tile_pool(name="wpool", bufs=1) as wpool,
            tc.tile_pool(name="hpool", bufs=1) as hpool,
            tc.tile_pool(name="spool", bufs=6) as spool,
            tc.tile_pool(name="psh", bufs=3, space="PSUM") as psh,
            tc.tile_pool(name="psy", bufs=5, space="PSUM") as psy,
        ):
            # PE warmup: dependency-light fp32 matmuls ramp the HAM clock to
            # 2.4 GHz while the first DMAs land. memset on the vector queue
            # starts earlier than gpsimd's.
            warm_sb = xpool.tile([128, 384], f32)
            nc.vector.memset(warm_sb, 0)
            for _ in range(8):
                pwarm = psy.tile([128, 512], f32, tag="py")
                nc.tensor.matmul(
                    pwarm[:, :256], warm_sb[:, :128], warm_sb[:, ds(128, 256)],
                    start=True, stop=True,
                )

            # xeT streams on the gpsimd (SWDGE) queue in 8 chunks so the
            # first GEMM1 groups can start after ~1 MB has landed; weights
            # stream on the sync (HWDGE) queue in parallel.
            xeT_sb = xpool.tile([128, N_DC, CAP], f16)
            for tcix in range(8):
                nc.gpsimd.dma_start(
                    out=xeT_sb[:, :, ds(tcix * 256, 256)],
                    in_=x_r[:, tcix, :, :],
                )

            w1_sb = wpool.tile([128, N_DC, F], f16, tag="w1")
            w2_sb = wpool.tile([128, F // 128, D], f16, tag="w2")
            # fo=0 weights first (dc-split so the first GEMM1 chain starts on
            # partial data), then interleave the rest by fo so each block's
            # weights land well before its compute.
            for dc in range(N_DC):
                nc.sync.dma_start(
                    out=w1_sb[:, dc, ds(0, F_BLK)],
                    in_=w1_r[:, 0, dc, :],
                )
            nc.sync.dma_start(out=w2_sb[:, ds(0, N_FC), :], in_=w2_r[:, 0, :, :])
            for fo in range(1, N_FBLK):
                nc.sync.dma_start(
                    out=w1_sb[:, :, ds(fo * F_BLK, F_BLK)],
                    in_=w1_r[:, fo, :, :],
                )
                nc.sync.dma_start(
                    out=w2_sb[:, ds(fo * N_FC, N_FC), :],
                    in_=w2_r[:, fo, :, :],
                )

            hT = hpool.tile([128, N_FC, CAP], f16)

            for fo in range(N_FBLK):
                # GEMM1: hT[f, t] = relu(sum_d W1[d, f] * XeT[d, t])
                # fo==0 walks 256-token columns to match streaming xeT
                # arrival; later blocks use full 512-wide moving operands.
                tok_cols = 2 * N_TC if fo == 0 else N_TC
                tok_w = CAP // tok_cols
                for tcix in range(tok_cols):
                    for fc in range(N_FC):
                        ph = psh.tile([128, 512], f32, tag="ph")
                        for dc in range(N_DC):
                            nc.tensor.matmul(
                                ph[:, :tok_w],
                                w1_sb[:, dc, ds(fo * F_BLK + fc * 128, 128)],
                                xeT_sb[:, dc, ds(tcix * tok_w, tok_w)],
                                start=(dc == 0),
                                stop=(dc == N_DC - 1),
                            )
                        nc.scalar.activation(
                            hT[:, fc, ds(tcix * tok_w, tok_w)],
                            ph[:, :tok_w],
                            mybir.ActivationFunctionType.Relu,
                        )

                # GEMM2: y_fo[t, d] = sum_f hT[f, t] * W2[f, d]; evacuate each
                # [128, 1024] token-group row as fp16 and stream it out on the
                # scalar (Activation) queue, summed across fo on host.
                for tg in range(N_TG):
                    stage = spool.tile([128, D], f16, tag="st")
                    for dh in range(2):
                        py = psy.tile([128, 512], f32, tag="py")
                        for fc in range(N_FC):
                            nc.tensor.matmul(
                                py,
                                hT[:, fc, ds(tg * 128, 128)],
                                w2_sb[:, fo * N_FC + fc, ds(dh * 512, 512)],
                                start=(fc == 0),
                                stop=(fc == N_FC - 1),
                            )
                        nc.vector.tensor_copy(stage[:, ds(dh * 512, 512)], py)
                    nc.scalar.dma_start(out=y_r[:, fo, tg, :], in_=stage)

    nc.compile()
    return nc


_NC = None


def _get_nc():
    global _NC
    if _NC is None:
        _NC = _build_nc()
    return _NC


def _route(xf, Wr):
    """Replicates the reference routing (jax-on-CPU, op-for-op) so that
    expert assignment matches the fp32 reference bit-exactly."""
    try:
        import jax
        import jax.numpy as jnp

        cpu = jax.local_devices(backend="cpu")[0]
        with jax.default_device(cpu):
            xj = jnp.asarray(xf, dtype=jnp.float32)
            wj = jnp.asarray(Wr, dtype=jnp.float32)
            probs = jax.nn.softmax(xj @ wj, axis=-1)
            eidx_j = jnp.argmax(probs, axis=-1)
            p_tok_j = jnp.take_along_axis(probs, eidx_j[:, None], axis=1)[:, 0]
            eidx = np.asarray(eidx_j)
            p_tok = np.asarray(p_tok_j)
    except Exception:
        # numpy fallback (fp32, same math; argmax ties broken identically
        # by first-max)
        logits = xf.astype(np.float32) @ Wr.astype(np.float32)
        lmax = logits.max(axis=-1, keepdims=True)
        ex = np.exp(logits - lmax)
        probs = ex / ex.sum(axis=-1, keepdims=True)
        eidx = np.argmax(probs, axis=-1)
        p_tok = probs[np.arange(T), eidx]

    # Integer capacity logic (exact) in numpy.
    onehot = np.zeros((T, E), dtype=np.int64)
    onehot[np.arange(T), eidx] = 1
    rank = np.cumsum(onehot, axis=0) - onehot
    rank = rank[np.arange(T), eidx]  # earlier same-expert tokens
    keep = rank < CAP

    dispatch = np.zeros((E, CAP), dtype=np.int64)
    valid = np.zeros((E, CAP), dtype=bool)
    kept = np.nonzero(keep)[0]
    dispatch[eidx[kept], rank[kept]] = kept
    valid[eidx[kept], rank[kept]] = True
    return dispatch, valid, p_tok


def kernel(x, Wr, W1, W2):
    from concourse.bass_utils import run_bass_kernel_spmd

    x = np.asarray(x, dtype=np.float32)
    Wr = np.asarray(Wr, dtype=np.float32)
    W1 = np.asarray(W1, dtype=np.float32)
    W2 = np.asarray(W2, dtype=np.float32)

    xf = x.reshape(T, D)
    dispatch, valid, p_tok = _route(xf, Wr)

    in_maps = []
    for e in range(E):
        scale = np.where(valid[e], p_tok[dispatch[e]], 0.0).astype(np.float32)
        xe = xf[dispatch[e]] * scale[:, None]  # [CAP, D]; relu(s*x@W1)@W2 = s*y
        xeT_t = xe.T.astype(np.float16).reshape(N_DC, 128, 8, 256).transpose(2, 1, 0, 3)
        w1_t = W1[e].astype(np.float16).reshape(N_DC, 128, N_FBLK, F_BLK).transpose(2, 1, 0, 3)
        w2_t = W2[e].astype(np.float16).reshape(N_FBLK, N_FC, 128, D).transpose(0, 2, 1, 3)
        in_maps.append({
            "xeT": np.ascontiguousarray(xeT_t),
            "w1": np.ascontiguousarray(w1_t),
            "w2": np.ascontiguousarray(w2_t),
        })

    nc = _get_nc()
    res = run_bass_kernel_spmd(nc, in_maps, core_ids=list(range(E)))

    yf = np.zeros((T, D), dtype=np.float32)
    for e in range(E):
        ye = res.results[e]["y"].astype(np.float32).sum(axis=0)  # [CAP, D]
        m = valid[e]
        yf[dispatch[e][m]] = ye[m]
    return yf.reshape(B, S, D)
